# revision 9
# baseline (speedup 1.0000x reference)
"""AgentGraphAttentionLayer (GAT layer, 4 heads) on 8 TRN2 NeuronCores.

Sharding: destination-node axis i across the 8 cores (512 rows each).
Every core computes the full Wx = x@W.T (replicated, small) plus its
row-block of attention scores / softmax / aggregation.

Per-core on-chip layout is [j(source) on partitions, i(dest) on free dim]:
  e[j,i,h]   = s_i[i,h] + s_j[j,h] + maskbias[j,i]   (maskbias: 0 / -30000)
  P[j,i,h]   = exp(leakyrelu(e))                      (masked lanes underflow to 0)
  out[i,c]  += P_h[:,i] . [Wx_h | 1][:,c]             (PE, 65th column = softmax denom)
then divide by the denominator, add the residual, LayerNorm, DMA out.
"""

import numpy as np
from contextlib import ExitStack

import concourse.bacc as bacc
import concourse.bass as bass
import concourse.tile as tile
import concourse.mybir as mybir
from concourse import bass_utils

N, F_IN, H, D = 4096, 256, 4, 64
OUTF = H * D  # 256
NCORES = 8
ISH = N // NCORES          # 512 destination rows per core
NJT = N // 128             # 32 j tiles
NIC = ISH // 128           # 4 i chunks
KB = F_IN // 128           # 2 contraction blocks
SLOPE = 0.2
MASKVAL = -30000.0
LN_EPS = 1e-5

f16 = mybir.dt.float16
bf16 = mybir.dt.bfloat16
f32 = mybir.dt.float32
ALU = mybir.AluOpType
ACTF = mybir.ActivationFunctionType

# leaky-relu engine split: True -> ScalarE (Prelu), False -> VectorE (mul+max)
def _lrelu_on_act(jt: int) -> bool:
    return jt % 5 < 3


def _rep4(ap):
    """[128, M] AP -> [128, 4, M] AP replicating the free dim 4x (step 0)."""
    return bass.AP(tensor=ap.tensor, offset=ap.offset,
                   ap=[ap.ap[0], [0, 4], ap.ap[1]])


def _bcast_rows(dram_ap, p, n):
    """1-D dram tensor [n] -> [p, n] AP broadcast along partitions."""
    return bass.AP(tensor=dram_ap.tensor, offset=dram_ap.offset,
                   ap=[[0, p], [1, n]])


def build_module():
    nc = bacc.Bacc("TRN2", target_bir_lowering=False, debug=False)
    adjb = nc.dram_tensor("adjb", [N, ISH], f16, kind="ExternalInput")
    xT = nc.dram_tensor("xT", [F_IN, N], f32, kind="ExternalInput")
    WT = nc.dram_tensor("WT", [F_IN, OUTF], f32, kind="ExternalInput")
    WA = nc.dram_tensor("WA", [F_IN, 2 * H], f32, kind="ExternalInput")
    xsh = nc.dram_tensor("xsh", [ISH, F_IN], f32, kind="ExternalInput")
    xshT = nc.dram_tensor("xshT", [F_IN, ISH], f32, kind="ExternalInput")
    gam = nc.dram_tensor("gam", [OUTF], f32, kind="ExternalInput")
    bet = nc.dram_tensor("bet", [OUTF], f32, kind="ExternalInput")
    out = nc.dram_tensor("out", [ISH, OUTF], f32, kind="ExternalOutput")

    with tile.TileContext(nc) as tc:
        with ExitStack() as ctx:
            consts = ctx.enter_context(tc.tile_pool(name="consts", bufs=1))
            stage = ctx.enter_context(tc.tile_pool(name="stage", bufs=3))
            stage2 = ctx.enter_context(tc.tile_pool(name="stage2", bufs=3))
            pstage = ctx.enter_context(tc.tile_pool(name="pstage", bufs=3))
            tailp = ctx.enter_context(tc.tile_pool(name="tailp", bufs=4))

            # ---- resident SBUF tensors ----
            adj_sb = consts.tile([128, NJT, ISH], f16)      # mask bias [j, i]
            xT_sb = consts.tile([128, KB, N], f32)
            WT_sb = consts.tile([128, KB, OUTF], f32)
            WA_sb = consts.tile([128, KB, 2 * H], f32)
            wxo_sb = consts.tile([128, NJT, H * 65], bf16)  # [Wx_h | ones] per head
            S_sb = consts.tile([128, NJT * 2 * H], f32)     # s_{src,dst}[n, .]
            srow_sb = [consts.tile([1, ISH], f16, name=f"srow{h}", tag=f"srow{h}")
                       for h in range(H)]                   # s_i rows (shard)
            sib_sb = consts.tile([128, H, ISH], f16)        # s_i broadcast per head
            xs_sb = consts.tile([128, NIC, F_IN], f32)
            gam_sb = consts.tile([128, OUTF], f32)
            bet_sb = consts.tile([128, OUTF], f32)
            ones_sb = consts.tile([1, 128], f16)
            eps_sb = consts.tile([128, 1], f32)
            nc.vector.memset(eps_sb, LN_EPS)

            for jt in range(NJT):
                nc.sync.dma_start(out=adj_sb[:, jt, :],
                                  in_=adjb.ap()[jt * 128:(jt + 1) * 128, :])
            for kb in range(KB):
                nc.sync.dma_start(out=xT_sb[:, kb, :],
                                  in_=xT.ap()[kb * 128:(kb + 1) * 128, :])
                nc.sync.dma_start(out=WT_sb[:, kb, :],
                                  in_=WT.ap()[kb * 128:(kb + 1) * 128, :])
                nc.sync.dma_start(out=WA_sb[:, kb, :],
                                  in_=WA.ap()[kb * 128:(kb + 1) * 128, :])
            nc.sync.dma_start(out=xs_sb,
                              in_=xsh.ap().rearrange("(c p) f -> p c f", p=128))
            nc.sync.dma_start(out=gam_sb, in_=_bcast_rows(gam.ap(), 128, OUTF))
            nc.sync.dma_start(out=bet_sb, in_=_bcast_rows(bet.ap(), 128, OUTF))
            nc.vector.memset(ones_sb, 1.0)
            # ones columns of wxo (col 64 of each head block); Wx copies leave them
            nc.vector.memset(wxo_sb, 1.0)

            # ---- phase 1: Wx, S (all n), S_shard.T, s_i broadcast ----
            with ExitStack() as c1:
                wx_ps = c1.enter_context(
                    tc.tile_pool(name="wx_ps", bufs=2, space="PSUM"))
                s_ps_pool = c1.enter_context(
                    tc.tile_pool(name="s_ps", bufs=1, space="PSUM"))
                bc_ps = c1.enter_context(
                    tc.tile_pool(name="bc_ps", bufs=1, space="PSUM"))

                s_ps = s_ps_pool.tile([128, NJT * 2 * H], f32)
                for nt in range(NJT):
                    wxp = wx_ps.tile([128, OUTF], f32)
                    for kb in range(KB):
                        lhsT = xT_sb[:, kb, nt * 128:(nt + 1) * 128]
                        nc.tensor.matmul(out=wxp, lhsT=lhsT, rhs=WT_sb[:, kb, :],
                                         start=(kb == 0), stop=(kb == KB - 1))
                        nc.tensor.matmul(out=s_ps[:, nt * 8:(nt + 1) * 8],
                                         lhsT=lhsT, rhs=WA_sb[:, kb, :],
                                         start=(kb == 0), stop=(kb == KB - 1))
                    # Wx (f32 psum) -> bf16 [head | ones] layout
                    dst = wxo_sb[:, nt, :].rearrange("p (h c) -> p h c", c=65)[:, :, 0:64]
                    src = wxp.rearrange("p (h d) -> p h d", d=64)
                    nc.vector.tensor_copy(out=dst, in_=src)
                nc.vector.tensor_copy(out=S_sb, in_=s_ps)

                # s_i rows of the shard: WA_src.T @ x_shard.T -> [1, ISH] per head
                xshT_sb = consts.tile([128, KB, ISH], f32)
                for kb in range(KB):
                    nc.sync.dma_start(out=xshT_sb[:, kb, :],
                                      in_=xshT.ap()[kb * 128:(kb + 1) * 128, :])
                for h in range(H):
                    sish_h = bc_ps.tile([1, ISH], f32, name=f"sish{h}",
                                        tag=f"sish{h}", bufs=1)
                    for kb in range(KB):
                        nc.tensor.matmul(out=sish_h,
                                         lhsT=WA_sb[:, kb, 2 * h:2 * h + 1],
                                         rhs=xshT_sb[:, kb, :],
                                         start=(kb == 0), stop=(kb == KB - 1))
                    nc.vector.tensor_copy(out=srow_sb[h], in_=sish_h)
                for h in range(H):
                    bcp = bc_ps.tile([128, ISH], f32)
                    nc.tensor.matmul(out=bcp, lhsT=ones_sb,
                                     rhs=srow_sb[h],
                                     start=True, stop=True)
                    nc.vector.tensor_copy(out=sib_sb[:, h, :], in_=bcp)

            # ---- phase 2: attention main loop ----
            with ExitStack() as c2:
                agg_pool = c2.enter_context(
                    tc.tile_pool(name="agg_ps", bufs=1, space="PSUM"))
                agg_ps = [agg_pool.tile([128, H * 65], f32, name=f"agg{i}", tag=f"agg{i}") for i in range(NIC)]

                for pair in range(NJT // 2):
                    eu = stage.tile([128, 2, H, ISH], f16)
                    pt = pstage.tile([128, 2, H, ISH], bf16)
                    for jt2 in range(2):
                        jt = pair * 2 + jt2
                        # e = adjbias (x4 heads) + s_i_bcast
                        nc.vector.tensor_tensor(
                            out=eu[:, jt2], in0=_rep4(adj_sb[:, jt, :]),
                            in1=sib_sb, op=ALU.add)
                        # += s_j (per-partition scalar, per head)
                        for h in range(H):
                            nc.vector.tensor_scalar_add(
                                out=eu[:, jt2, h, :], in0=eu[:, jt2, h, :],
                                scalar1=S_sb[:, jt * 8 + 2 * h + 1:jt * 8 + 2 * h + 2])
                        # leaky relu (in place)
                        if _lrelu_on_act(jt):
                            nc.scalar.activation(out=eu[:, jt2], in_=eu[:, jt2],
                                                 func=ACTF.Prelu, alpha=SLOPE)
                        else:
                            tmp = stage2.tile([128, H, ISH], f16)
                            nc.vector.tensor_scalar_mul(
                                out=tmp, in0=eu[:, jt2], scalar1=SLOPE)
                            nc.vector.tensor_tensor(
                                out=eu[:, jt2], in0=eu[:, jt2], in1=tmp, op=ALU.max)
                        # P = exp(.)
                        nc.scalar.activation(out=pt[:, jt2], in_=eu[:, jt2],
                                             func=ACTF.Exp)
                        # aggregate
                        for h in range(H):
                            for ic in range(NIC):
                                nc.tensor.matmul(
                                    out=agg_ps[ic][:, h * 65:(h + 1) * 65],
                                    lhsT=pt[:, jt2, h, ic * 128:(ic + 1) * 128],
                                    rhs=wxo_sb[:, jt, h * 65:(h + 1) * 65],
                                    start=(jt == 0), stop=(jt == NJT - 1))

                # ---- phase 3: normalize + residual + LayerNorm ----
                for ic in range(NIC):
                    den = tailp.tile([128, H], f32)
                    nc.vector.tensor_copy(
                        out=den,
                        in_=agg_ps[ic].rearrange("p (h c) -> p h c", c=65)[:, :, 64])
                    rden = tailp.tile([128, H], f32)
                    nc.vector.reciprocal(rden, den)
                    o_sb = tailp.tile([128, OUTF], f32)
                    for h in range(H):
                        nc.vector.tensor_scalar_mul(
                            out=o_sb[:, h * 64:(h + 1) * 64],
                            in0=agg_ps[ic][:, h * 65:h * 65 + 64],
                            scalar1=rden[:, h:h + 1])
                    nc.vector.tensor_tensor(out=o_sb, in0=o_sb,
                                            in1=xs_sb[:, ic, :], op=ALU.add)
                    stats = tailp.tile([128, 6], f32)
                    nc.vector.bn_stats(out=stats, in_=o_sb)
                    mv = tailp.tile([128, 2], f32)
                    nc.vector.bn_aggr(out=mv, in_=stats)
                    std = tailp.tile([128, 1], f32)
                    nc.scalar.activation(out=std, in_=mv[:, 1:2], func=ACTF.Sqrt,
                                         bias=eps_sb[:, 0:1])
                    rstd = tailp.tile([128, 1], f32)
                    nc.vector.reciprocal(rstd, std)
                    nc.vector.tensor_scalar(out=o_sb, in0=o_sb,
                                            scalar1=mv[:, 0:1], scalar2=rstd,
                                            op0=ALU.subtract, op1=ALU.mult)
                    nc.vector.tensor_tensor(out=o_sb, in0=o_sb, in1=gam_sb,
                                            op=ALU.mult)
                    nc.vector.tensor_tensor(out=o_sb, in0=o_sb, in1=bet_sb,
                                            op=ALU.add)
                    nc.sync.dma_start(out=out.ap()[ic * 128:(ic + 1) * 128, :],
                                      in_=o_sb)
    nc.compile()
    return nc


_CACHE = {}


def _get_module():
    if "nc" not in _CACHE:
        _CACHE["nc"] = build_module()
    return _CACHE["nc"]


def kernel(x, adjacency, W, a, ln_gamma, ln_beta):
    x = np.asarray(x, dtype=np.float32)
    adjacency = np.asarray(adjacency)
    W = np.asarray(W, dtype=np.float32)
    a = np.asarray(a, dtype=np.float32)
    ln_gamma = np.asarray(ln_gamma, dtype=np.float32)
    ln_beta = np.asarray(ln_beta, dtype=np.float32)

    xT = np.ascontiguousarray(x.T)
    WT = np.ascontiguousarray(W.T)
    WA = np.zeros((F_IN, 2 * H), np.float32)
    for h in range(H):
        Wh = W[h * D:(h + 1) * D, :]            # [D, F_IN]
        WA[:, 2 * h] = Wh.T @ a[h, :D]          # a_src -> s_i
        WA[:, 2 * h + 1] = Wh.T @ a[h, D:]      # a_dst -> s_j
    adjb_full = np.where(adjacency.T, np.float16(0.0),
                         np.float16(MASKVAL))   # [j, i]

    nc = _get_module()
    in_maps = []
    for c in range(NCORES):
        xs_c = np.ascontiguousarray(x[c * ISH:(c + 1) * ISH, :])
        in_maps.append({
            "adjb": np.ascontiguousarray(adjb_full[:, c * ISH:(c + 1) * ISH]),
            "xT": xT,
            "WT": WT,
            "WA": WA,
            "xsh": xs_c,
            "xshT": np.ascontiguousarray(xs_c.T),
            "gam": ln_gamma,
            "bet": ln_beta,
        })
    res = bass_utils.run_bass_kernel_spmd(nc, in_maps,
                                          core_ids=list(range(NCORES)))
    return np.concatenate([res.results[c]["out"] for c in range(NCORES)], axis=0)


# revision 14
# speedup vs baseline: 1.1568x; 1.1568x over previous
"""AgentGraphAttentionLayer (GAT layer, 4 heads) on 8 TRN2 NeuronCores.

Sharding: destination-node axis i across the 8 cores (512 rows each).
Every core computes the full Wx = x@W.T (replicated, small) plus its
row-block of attention scores / softmax / aggregation.

Per-core on-chip layout is [j(source) on partitions, i(dest) on free dim]:
  e[j,i,h]   = s_i[i,h] + s_j[j,h] + maskbias[j,i]   (maskbias: 0 / -30000)
  P[j,i,h]   = exp(leakyrelu(e))                      (masked lanes underflow to 0)
  psum_h[c,i] += [Wx_h | 1][j,c] . P_h[j,i]           (PE, row 64 = softmax denom)
then transpose back to [i, c], divide by the denominator, residual, LayerNorm.

e+mask+leakyrelu is one fused custom DVE op (registered at import time).
"""

import numpy as np
from contextlib import ExitStack

import concourse.bacc as bacc
import concourse.bass as bass
import concourse.tile as tile
import concourse.mybir as mybir
from concourse import bass_utils
from concourse import dve_ops as _dvo
from concourse.dve_spec import Spec, Src0, Src1, C0, C1, maxx, lower
from concourse.dve_uop import DveOpSpec

N, F_IN, H, D = 4096, 256, 4, 64
OUTF = H * D  # 256
NCORES = 8
ISH = N // NCORES          # 512 destination rows per core
NJT = N // 128             # 32 j tiles
NIC = ISH // 128           # 4 i chunks
KB = F_IN // 128           # 2 contraction blocks
SLOPE = 0.2
MASKVAL = -30000.0
LN_EPS = 1e-5

f16 = mybir.dt.float16
bf16 = mybir.dt.bfloat16
f32 = mybir.dt.float32
ALU = mybir.AluOpType
ACTF = mybir.ActivationFunctionType


def _build_2x_uop():
    """Hand-authored 2X_1PORT program: two parallel 4-stage leakyrelu chains
    (lo element in blocks 0-3, hi element in blocks 4-7).

    input lanes -> block-0 delay chains: D0=S0 D1=S1 D2=C0 D3=C1 D4=S0H D5=S1H
    """
    from concourse.dve_uop import (UopConfig, UopDpConfig, InpSel, AluInp,
                                   AluOp, DelayInp, OutPath, OutSel, Trigger)
    blocks = []

    def blk():
        b = UopDpConfig()
        blocks.append(b)
        return b

    b0 = blk().enable_alu(AluOp.ADD, AluInp.PREV_DELAY_0, AluInp.PREV_DELAY_1)
    b0.pass_through_delay(2, 3, 4, 5)
    b1 = blk().enable_alu(AluOp.ADD, AluInp.PREV_ALU_OUT, AluInp.PREV_DELAY_2)
    b1.pass_through_delay(2, 3, 4, 5)
    b2 = blk().enable_alu(AluOp.MULTIPLY, AluInp.PREV_ALU_OUT,
                          AluInp.PREV_DELAY_3)
    b2.enable_delay_from_src(DelayInp.PREV_ALU_OUT, 0)   # D0 <- w_lo
    b2.pass_through_delay(2, 3, 4, 5)
    b3 = blk().enable_alu(AluOp.MAX, AluInp.PREV_DELAY_0, AluInp.PREV_ALU_OUT)
    b3.pass_through_delay(2, 3, 4, 5)
    b4 = blk().enable_alu(AluOp.ADD, AluInp.PREV_DELAY_4, AluInp.PREV_DELAY_5)
    b4.enable_delay_from_src(DelayInp.PREV_ALU_OUT, 0)   # D0 <- res_lo
    b4.pass_through_delay(2, 3)
    b5 = blk().enable_alu(AluOp.ADD, AluInp.PREV_ALU_OUT, AluInp.PREV_DELAY_2)
    b5.pass_through_delay(0, 3)
    b6 = blk().enable_alu(AluOp.MULTIPLY, AluInp.PREV_ALU_OUT,
                          AluInp.PREV_DELAY_3)
    b6.enable_delay_from_src(DelayInp.PREV_ALU_OUT, 1)   # D1 <- w_hi
    b6.pass_through_delay(0)
    b7 = blk().enable_alu(AluOp.MAX, AluInp.PREV_DELAY_1, AluInp.PREV_ALU_OUT)
    b7.pass_through_delay(0)
    return UopConfig(
        inp=[InpSel.ZERO, InpSel.SRC_0, InpSel.SRC_1, InpSel.CONST_0,
             InpSel.CONST_1, InpSel.SRC_0_HI, InpSel.SRC_1_HI, InpSel.ZERO],
        inp_enable=[0, 1, 1, 1, 1, 1, 1, 0],
        out={OutPath.WR0_LO: OutSel.DELAY_0, OutPath.WR0_HI: OutSel.ALU_OUT,
             OutPath.WR1_LO: OutSel.ALU_OUT, OutPath.WR1_HI: OutSel.ALU_OUT},
        out_enable={OutPath.WR0_LO: 1, OutPath.WR0_HI: 1,
                    OutPath.WR1_LO: 0, OutPath.WR1_HI: 0},
        require_inp0=1, require_inp1=1,
        trigger=(Trigger.SRC_TENSOR_DONE, Trigger.NONE, Trigger.NONE),
        next_uop=(0, 0, 0),
        datapath_config=blocks)


def _register_lrelu_op():
    """out = max(w, w*s1) with w = in0 + in1 + s0  (leakyrelu of biased sum).

    A hand-authored 2X_1PORT uop variant is injected via the compile cache;
    instructions opt in with perf_max=1 (hw falls back to 1x if ineligible).
    """
    name = "LRELU_MASK_EMB_ANT"
    for op in _dvo.OPS:
        if op.name == name:
            return op
    w = Src0 + Src1 + C0
    spec = Spec(
        body=maxx(w, w * C1),
        reference=lambda in0, in1, s0, s1, imm2: (
            lambda ww: np.maximum(ww, ww * np.float32(s1)).astype(np.float32)
        )(in0.astype(np.float32) + in1.astype(np.float32)
          + np.asarray(s0, np.float32).reshape(-1, 1)),
    )
    shas = {}
    for ver in ("v3", "v4"):
        tmp = DveOpSpec(name=name, opcode=0, uops=lower(spec, ver=ver),
                        rd1_en=True)
        shas[ver] = tmp.sha(ver)
    op = _dvo.DveOp(name, spec, subdim=False, uops_sha=shas)
    _dvo.OPS.append(op)
    _dvo.CUSTOM_DVE_SPECS[name] = spec
    _dvo._SUB_OPCODE_FOR_NAME[name] = _dvo._CUSTOM_DVE_ROW_BASE + len(_dvo.OPS) - 1
    row = _dvo._SUB_OPCODE_FOR_NAME[name]
    assert row < 0x20
    # pre-seed the compile cache with the 2x-capable spec (v3 = TRN2)
    spec3 = DveOpSpec(name=name, opcode=row, uops=lower(spec, ver="v3"),
                      uops_2x=[_build_2x_uop()], perf_max=1, rd1_en=True)
    spec3.validate("v3")
    _dvo._COMPILE_CACHE[(name, "v3")] = spec3
    return op


LRELU_OP = _register_lrelu_op()


def _rep4(ap):
    """[128, M] AP -> [128, 4, M] AP replicating the free dim 4x (step 0)."""
    return bass.AP(tensor=ap.tensor, offset=ap.offset,
                   ap=[ap.ap[0], [0, 4], ap.ap[1]])


def _bcast_rows(dram_ap, p, n):
    """1-D dram tensor [n] -> [p, n] AP broadcast along partitions."""
    return bass.AP(tensor=dram_ap.tensor, offset=dram_ap.offset,
                   ap=[[0, p], [1, n]])


def build_module():
    nc = bacc.Bacc("TRN2", target_bir_lowering=False, debug=False)
    # adjb comes pre-swizzled: adjb[p, jt*ISH + i] = maskbias[jt*128+p, i]
    adjb = nc.dram_tensor("adjb", [128, NJT * ISH], f16, kind="ExternalInput")
    xT = nc.dram_tensor("xT", [F_IN, N], f32, kind="ExternalInput")
    WT = nc.dram_tensor("WT", [F_IN, OUTF], f32, kind="ExternalInput")
    WA = nc.dram_tensor("WA", [F_IN, 2 * H], f32, kind="ExternalInput")
    xsh = nc.dram_tensor("xsh", [ISH, F_IN], f32, kind="ExternalInput")
    xshT = nc.dram_tensor("xshT", [F_IN, ISH], f32, kind="ExternalInput")
    eye = nc.dram_tensor("eye", [128, 128], f32, kind="ExternalInput")
    gam = nc.dram_tensor("gam", [OUTF], f32, kind="ExternalInput")
    bet = nc.dram_tensor("bet", [OUTF], f32, kind="ExternalInput")
    out = nc.dram_tensor("out", [ISH, OUTF], f32, kind="ExternalOutput")

    with tile.TileContext(nc) as tc:
        with ExitStack() as ctx:
            consts = ctx.enter_context(tc.tile_pool(name="consts", bufs=1))
            stage = ctx.enter_context(tc.tile_pool(name="stage", bufs=3))
            pstage = ctx.enter_context(tc.tile_pool(name="pstage", bufs=3))
            tailp = ctx.enter_context(tc.tile_pool(name="tailp", bufs=4))

            # ---- resident SBUF tensors ----
            adj_sb = consts.tile([128, NJT, ISH], f16)  # mask bias 0 / -30000
            xT_sb = consts.tile([128, KB, N], f32)
            WT_sb = consts.tile([128, KB, OUTF], f32)
            WA_sb = consts.tile([128, KB, 2 * H], f32)
            wxo_sb = consts.tile([128, NJT, H * 65], bf16)  # [Wx_h | ones] per head
            S_sb = consts.tile([128, NJT * 2 * H], f32)     # s_{src,dst}[n, .]
            srow_sb = [consts.tile([1, ISH], f16, name=f"srow{h}", tag=f"srow{h}")
                       for h in range(H)]                   # s_i rows (shard)
            sib_sb = consts.tile([128, H, ISH], f16)        # s_i broadcast per head
            xs_sb = consts.tile([128, NIC, F_IN], f32)
            eye_sb = consts.tile([128, 128], f32)
            gam_sb = consts.tile([128, OUTF], f32)
            bet_sb = consts.tile([128, OUTF], f32)
            ones_sb = consts.tile([1, 128], f16)
            eps_sb = consts.tile([128, 1], f32)
            nc.vector.memset(eps_sb, LN_EPS)

            for q in range(4):  # adjacency in 4 big contiguous chunks
                w = NJT * ISH // 4
                nc.sync.dma_start(out=adj_sb[:, q * 8:(q + 1) * 8, :],
                                  in_=adjb.ap()[:, q * w:(q + 1) * w])
            for kb in range(KB):
                nc.sync.dma_start(out=xT_sb[:, kb, :],
                                  in_=xT.ap()[kb * 128:(kb + 1) * 128, :])
                nc.sync.dma_start(out=WT_sb[:, kb, :],
                                  in_=WT.ap()[kb * 128:(kb + 1) * 128, :])
                nc.sync.dma_start(out=WA_sb[:, kb, :],
                                  in_=WA.ap()[kb * 128:(kb + 1) * 128, :])
            nc.sync.dma_start(out=xs_sb,
                              in_=xsh.ap().rearrange("(c p) f -> p c f", p=128))
            nc.sync.dma_start(out=eye_sb, in_=eye.ap())
            nc.sync.dma_start(out=gam_sb, in_=_bcast_rows(gam.ap(), 128, OUTF))
            nc.sync.dma_start(out=bet_sb, in_=_bcast_rows(bet.ap(), 128, OUTF))
            nc.vector.memset(ones_sb, 1.0)
            # ones columns (col 64 of each head block of wxo)
            ones_cols = wxo_sb.rearrange("p t (h c) -> p t h c", c=65)[:, :, :, 64]
            nc.vector.memset(ones_cols, 1.0)

            # ---- phase 1: Wx, S (all n), s_i rows (shard), s_i broadcast ----
            with ExitStack() as c1:
                wx_ps = c1.enter_context(
                    tc.tile_pool(name="wx_ps", bufs=2, space="PSUM"))
                s_ps_pool = c1.enter_context(
                    tc.tile_pool(name="s_ps", bufs=1, space="PSUM"))
                bc_ps = c1.enter_context(
                    tc.tile_pool(name="bc_ps", bufs=1, space="PSUM"))

                s_ps = s_ps_pool.tile([128, NJT * 2 * H], f32)
                for nt in range(NJT):
                    wxp = wx_ps.tile([128, OUTF], f32)
                    for kb in range(KB):
                        lhsT = xT_sb[:, kb, nt * 128:(nt + 1) * 128]
                        nc.tensor.matmul(out=wxp, lhsT=lhsT, rhs=WT_sb[:, kb, :],
                                         start=(kb == 0), stop=(kb == KB - 1))
                        nc.tensor.matmul(out=s_ps[:, nt * 8:(nt + 1) * 8],
                                         lhsT=lhsT, rhs=WA_sb[:, kb, :],
                                         start=(kb == 0), stop=(kb == KB - 1))
                    # Wx (f32 psum) -> bf16 [head | ones] layout (on ScalarE)
                    dst = wxo_sb[:, nt, :].rearrange("p (h c) -> p h c", c=65)[:, :, 0:64]
                    src = wxp.rearrange("p (h d) -> p h d", d=64)
                    nc.scalar.copy(out=dst, in_=src)
                nc.vector.tensor_copy(out=S_sb, in_=s_ps)

                # s_i rows of the shard: WA_src.T @ x_shard.T -> [1, ISH] per head
                xshT_sb = consts.tile([128, KB, ISH], f32)
                for kb in range(KB):
                    nc.sync.dma_start(out=xshT_sb[:, kb, :],
                                      in_=xshT.ap()[kb * 128:(kb + 1) * 128, :])
                for h in range(H):
                    sish_h = bc_ps.tile([1, ISH], f32, name=f"sish{h}",
                                        tag=f"sish{h}", bufs=1)
                    for kb in range(KB):
                        nc.tensor.matmul(out=sish_h,
                                         lhsT=WA_sb[:, kb, 2 * h:2 * h + 1],
                                         rhs=xshT_sb[:, kb, :],
                                         start=(kb == 0), stop=(kb == KB - 1))
                    nc.vector.tensor_copy(out=srow_sb[h], in_=sish_h)
                for h in range(H):
                    bcp = bc_ps.tile([128, ISH], f32)
                    nc.tensor.matmul(out=bcp, lhsT=ones_sb, rhs=srow_sb[h],
                                     start=True, stop=True)
                    nc.vector.tensor_copy(out=sib_sb[:, h, :], in_=bcp)

            # ---- phase 2: attention main loop ----
            with ExitStack() as c2:
                agg_pool = c2.enter_context(
                    tc.tile_pool(name="agg_ps", bufs=1, space="PSUM"))
                tp_ps_pool = c2.enter_context(
                    tc.tile_pool(name="tp_ps", bufs=2, space="PSUM"))
                agg_ps = [agg_pool.tile([65, ISH], f32, name=f"agg{h}",
                                        tag=f"agg{h}") for h in range(H)]

                for pair in range(NJT // 2):
                    eu = stage.tile([128, 2, H, ISH], f16)
                    pt = pstage.tile([128, 2, H, ISH], bf16)
                    for jt2 in range(2):
                        jt = pair * 2 + jt2
                        for h in range(H):
                            sj = S_sb[:, jt * 8 + 2 * h + 1:jt * 8 + 2 * h + 2]
                            ci = nc.vector._custom_dve(
                                LRELU_OP, out=eu[:, jt2, h, :],
                                in0=sib_sb[:, h, :], in1=adj_sb[:, jt, :],
                                s0=sj, s1=SLOPE)
                            ci.ins.perf_max = 1
                    # P = exp(.) for the whole pair
                    nc.scalar.activation(out=pt, in_=eu, func=ACTF.Exp)
                    for jt2 in range(2):
                        jt = pair * 2 + jt2
                        for h in range(H):
                            nc.tensor.matmul(
                                out=agg_ps[h],
                                lhsT=wxo_sb[:, jt, h * 65:(h + 1) * 65],
                                rhs=pt[:, jt2, h, :],
                                start=(jt == 0), stop=(jt == NJT - 1))

                # ---- phase 3: transpose, normalize, residual, LayerNorm ----
                oc_sb = consts.tile([65, H, ISH], f32)
                for h in range(H):
                    nc.scalar.copy(out=oc_sb[:, h, :], in_=agg_ps[h])
                ot_sb = consts.tile([128, NIC, H * 65], f32)
                for ic in range(NIC):
                    tpp = tp_ps_pool.tile([128, H * 65], f32)
                    for h in range(H):
                        nc.tensor.transpose(
                            out=tpp[:, h * 65:(h + 1) * 65],
                            in_=oc_sb[:, h, ic * 128:(ic + 1) * 128],
                            identity=eye_sb[0:65, 0:65])
                    nc.vector.tensor_copy(out=ot_sb[:, ic, :], in_=tpp)
                for ic in range(NIC):
                    den = tailp.tile([128, H], f32)
                    nc.vector.tensor_copy(
                        out=den,
                        in_=ot_sb[:, ic, :].rearrange(
                            "p (h c) -> p h c", c=65)[:, :, 64])
                    rden = tailp.tile([128, H], f32)
                    nc.vector.reciprocal(rden, den)
                    o_sb = tailp.tile([128, OUTF], f32)
                    for h in range(H):
                        nc.vector.tensor_scalar_mul(
                            out=o_sb[:, h * 64:(h + 1) * 64],
                            in0=ot_sb[:, ic, h * 65:h * 65 + 64],
                            scalar1=rden[:, h:h + 1])
                    nc.vector.tensor_tensor(out=o_sb, in0=o_sb,
                                            in1=xs_sb[:, ic, :], op=ALU.add)
                    stats = tailp.tile([128, 6], f32)
                    nc.vector.bn_stats(out=stats, in_=o_sb)
                    mv = tailp.tile([128, 2], f32)
                    nc.vector.bn_aggr(out=mv, in_=stats)
                    std = tailp.tile([128, 1], f32)
                    nc.scalar.activation(out=std, in_=mv[:, 1:2], func=ACTF.Sqrt,
                                         bias=eps_sb[:, 0:1])
                    rstd = tailp.tile([128, 1], f32)
                    nc.vector.reciprocal(rstd, std)
                    nc.vector.tensor_scalar(out=o_sb, in0=o_sb,
                                            scalar1=mv[:, 0:1], scalar2=rstd,
                                            op0=ALU.subtract, op1=ALU.mult)
                    nc.vector.tensor_tensor(out=o_sb, in0=o_sb, in1=gam_sb,
                                            op=ALU.mult)
                    nc.vector.tensor_tensor(out=o_sb, in0=o_sb, in1=bet_sb,
                                            op=ALU.add)
                    nc.sync.dma_start(out=out.ap()[ic * 128:(ic + 1) * 128, :],
                                      in_=o_sb)
    nc.compile()
    return nc


_CACHE = {}


def _get_module():
    if "nc" not in _CACHE:
        _CACHE["nc"] = build_module()
    return _CACHE["nc"]


def kernel(x, adjacency, W, a, ln_gamma, ln_beta):
    x = np.asarray(x, dtype=np.float32)
    adjacency = np.asarray(adjacency)
    W = np.asarray(W, dtype=np.float32)
    a = np.asarray(a, dtype=np.float32)
    ln_gamma = np.asarray(ln_gamma, dtype=np.float32)
    ln_beta = np.asarray(ln_beta, dtype=np.float32)

    xT = np.ascontiguousarray(x.T)
    WT = np.ascontiguousarray(W.T)
    WA = np.zeros((F_IN, 2 * H), np.float32)
    for h in range(H):
        Wh = W[h * D:(h + 1) * D, :]            # [D, F_IN]
        WA[:, 2 * h] = Wh.T @ a[h, :D]          # a_src -> s_i
        WA[:, 2 * h + 1] = Wh.T @ a[h, D:]      # a_dst -> s_j
    adjb_full = np.where(adjacency.T, np.float16(0.0),
                         np.float16(MASKVAL))   # [j, i]
    eye = np.eye(128, dtype=np.float32)

    nc = _get_module()
    in_maps = []
    for c in range(NCORES):
        xs_c = np.ascontiguousarray(x[c * ISH:(c + 1) * ISH, :])
        adjb_c = adjb_full[:, c * ISH:(c + 1) * ISH]
        adjb_c = np.ascontiguousarray(
            adjb_c.reshape(NJT, 128, ISH).transpose(1, 0, 2).reshape(
                128, NJT * ISH))
        in_maps.append({
            "adjb": adjb_c,
            "xT": xT,
            "WT": WT,
            "WA": WA,
            "xsh": xs_c,
            "xshT": np.ascontiguousarray(xs_c.T),
            "eye": eye,
            "gam": ln_gamma,
            "bet": ln_beta,
        })
    res = bass_utils.run_bass_kernel_spmd(nc, in_maps,
                                          core_ids=list(range(NCORES)))
    return np.concatenate([res.results[c]["out"] for c in range(NCORES)], axis=0)
